# revision 55
# baseline (speedup 1.0000x reference)
"""Trainium2 Bass kernel for a TF-style GRU + sigmoid projection.

Reference computation (B=32, T=2048, D=H=OUT=256):
    ru  = sigmoid([x_t, h] @ Wg + bg);  r, u = split(ru)
    c   = tanh([x_t, r*h] @ Wc + bc)
    h'  = u*h + (1-u)*c
    out = sigmoid(H @ Wp + bp)          # H = all h_t

Strategy: data-parallel over batch (8 cores x 4 sequences), and
parallel-in-time inside each core via fixed-point (quasi-DEER) sweeps:

    sweep k:  for ALL t in parallel (big matmuls, full engine occupancy):
                  pr,pu = Gx_t + Wgh @ h^{k-1}_{t-1};  r,u = sigmoid
                  c     = tanh(Cx_t + Wch @ (r * h^{k-1}_{t-1}))
                  z     = (u-1)*c            # -(1-u)*c
              then one hardware prefix scan per (k-tile, seq):
                  h^k_t = u_t * h^k_{t-1} - z_t     (tensor_tensor_scan)

The scan makes the u-memory chain exact every sweep; only the gate/candidate
coupling iterates, contracting ~0.37x per sweep.  K=3 sweeps reach ~5e-3
rel L2 (gate is 2e-2).  Sweep 1 (h=0) doubles as the Gx/Cx staging pass.

Scheduling notes:
  - The two 4.4us serial scans per (sweep, seq) are DEFERRED into the next
    seq-block's instruction stream (after chunks 2 and 6) so the in-order
    DVE queue never head-of-line-blocks the rh products the PE is waiting
    on.  Projection of block b runs right after b's second deferred scan.
  - z = (u-1)*c is batched over ZBLK columns to amortize DVE overhead.
  - Sweep 1 issues two throwaway 512-col identity matmuls per chunk to keep
    the otherwise ACT-paced PE stream dense enough to hold full clock
    (TRN2 PE drops to 1.2GHz when its busy streak breaks).

Everything on chip is hidden-major: [128 partitions = half the hidden dim,
2 k-tiles, cols] with col = seq*2048 + t (t fastest, so the scan can run
along the free dimension per sequence).
"""

import numpy as np

B, T, D = 32, 2048, 256
H, OUT = 256, 256
NCORES = 8
BLOC = B // NCORES      # 4 sequences per core
N = T * BLOC            # 8192 cols, col = b*T + t
CH = 256                # cols per psum chunk
CPB = T // CH           # chunks per sequence
XBLK = 1024             # x-stream DMA block
OBLK = 256              # output DMA block
ZBLK = 1024             # cols per batched z (stt) op
K = 3                   # fixed-point sweeps

# packed-weight column offsets (bf16).  wpk1 = sweep-1 set, wpk2 = the rest.
PK1_WGX = 0       # [2k x 512]
PK1_WCX = 1024    # [2k x 256]
PK1_EYE = 1536    # [128]
PK1_BG01 = 1664   # rows 0-1: bg[m*128+p] for m=0,1 (transposed bias)
PK1_BG23 = 1792   # rows 0-1: bg[(2+m)*128+p]
PK1_BC = 1920     # rows 0-1: bc[m*128+p]
PK1_MASK = 2048   # rows 0-1: one-hot [2 x 2*CH] (mask[r, m*CH+cc] = r==m)
PKW1 = 2560
PK2_WGH = 0       # [2k x 512]
PK2_WCH = 1024    # [2k x 256]
PK2_WP = 1536     # [2k x 256]
PK2_BP = 2048     # cols 2048+mo: bp[mo*128+p] as [128,1] columns
PKW2 = 2050

_cache = {}


def _build(K_, CH_):
    import concourse.bacc as bacc
    import concourse.mybir as mybir
    from concourse.tile import TileContext

    f32 = mybir.dt.float32
    bf16 = mybir.dt.bfloat16
    AF = mybir.ActivationFunctionType
    ALU = mybir.AluOpType

    CPB_ = T // CH_
    PBLK = XBLK // CH_   # chunks per x DMA block
    OPB = OBLK // CH_    # chunks per out DMA block
    ZPB = ZBLK // CH_    # chunks per batched z op

    nc = bacc.Bacc("TRN2", target_bir_lowering=False, debug=False)

    xT_d = nc.declare_dram_parameter("xT", [2, 128, N], bf16, isOutput=False)
    wpk1_d = nc.declare_dram_parameter("wpk1", [128, PKW1], bf16,
                                       isOutput=False)
    wpk2_d = nc.declare_dram_parameter("wpk2", [128, PKW2], bf16,
                                       isOutput=False)
    outT_d = nc.declare_dram_parameter("outT", [128, 2, N], f32, isOutput=True)

    with TileContext(nc) as tc:
        with (
            tc.tile_pool(name="const", bufs=1) as const,
            tc.tile_pool(name="xc", bufs=2) as xcp,
            tc.tile_pool(name="csc", bufs=2) as csc,
            tc.tile_pool(name="rhsc", bufs=2) as rhsc,
            tc.tile_pool(name="rub", bufs=2) as rubp,
            tc.tile_pool(name="ob", bufs=2) as obp,
            tc.tile_pool(name="psg", bufs=2, space="PSUM") as psg,
            tc.tile_pool(name="psc", bufs=2, space="PSUM") as psc,
            tc.tile_pool(name="psp", bufs=2, space="PSUM") as psp,
        ):
            gx = const.tile([128, 4, N], bf16)   # Gx+bg, m = [r0,r1,u0,u1]
            cx = const.tile([128, 2, N], bf16)   # Cx+bc
            h = const.tile([128, 2, N], bf16)
            w1 = const.tile([128, PKW1], bf16)
            w2 = const.tile([128, PKW2], bf16)

            # boot DMAs on separate engine queues so the transfers overlap;
            # the small eye/bias/mask range lands first so the first chunk's
            # bias matmuls can issue while wgx/wcx stream in
            xc0 = xcp.tile([128, 2, XBLK], bf16, tag="xc")
            nc.sync.dma_start(out=w1[:, PK1_EYE:], in_=wpk1_d[:, PK1_EYE:])
            for k in range(2):
                nc.scalar.dma_start(out=xc0[:, k, :], in_=xT_d[k, :, 0:XBLK])
            nc.sync.dma_start(out=w1[:, :PK1_EYE], in_=wpk1_d[:, :PK1_EYE])
            nc.gpsimd.dma_start(out=w2[:], in_=wpk2_d[:])
            # one-hot mask moving operand: broadcasts a transposed bias row
            # into both m-tiles of a psum tile with ONE 2-partition matmul
            # (1-partition `ones` moving operands cost a PE pipeline drain)
            mask2 = w1[0:2, PK1_MASK:PK1_MASK + 2 * CH_]

            def wgx(k, m):
                return w1[:, PK1_WGX + k * 512 + m * 128:
                          PK1_WGX + k * 512 + (m + 1) * 128]

            def wcx(k, m):
                return w1[:, PK1_WCX + k * 256 + m * 128:
                          PK1_WCX + k * 256 + (m + 1) * 128]

            def wgh(k, m):
                return w2[:, PK2_WGH + k * 512 + m * 128:
                          PK2_WGH + k * 512 + (m + 1) * 128]

            def wch(k, m):
                return w2[:, PK2_WCH + k * 256 + m * 128:
                          PK2_WCH + k * 256 + (m + 1) * 128]

            def wp(k, m):
                return w2[:, PK2_WP + k * 256 + m * 128:
                          PK2_WP + k * 256 + (m + 1) * 128]

            eye = w1[:, PK1_EYE:PK1_EYE + 128]

            def sweep1_chunk(b, j, xc, rub_t, cb):
                """pg/pc = x-part + bias; store Gx/Cx; u, c for the scan."""
                s = b * T + j * CH_
                off = (j % PBLK) * CH_
                co = j * CH_
                jsl = slice(j * CH_, (j + 1) * CH_)
                pg = psg.tile([128, 4, CH_], f32, tag="pg")
                pc = psc.tile([128, 2, CH_], f32, tag="pc")
                # clustered transposed-bias matmuls first (start=True resets
                # each bank); all three share the mask2 moving operand so the
                # PE pays at most one moving-partition-count transition
                nc.tensor.matmul(
                    pg[:, 0:2, :], w1[0:2, PK1_BG01:PK1_BG01 + 128],
                    mask2, start=True, stop=False)
                nc.tensor.matmul(
                    pg[:, 2:4, :], w1[0:2, PK1_BG23:PK1_BG23 + 128],
                    mask2, start=True, stop=False)
                nc.tensor.matmul(
                    pc[:, :, :], w1[0:2, PK1_BC:PK1_BC + 128],
                    mask2, start=True, stop=False)
                for m in range(4):
                    for k in range(2):
                        nc.tensor.matmul(
                            pg[:, m, :], wgx(k, m), xc[:, k, off:off + CH_],
                            start=False, stop=(k == 1),
                        )
                for m in range(2):
                    for k in range(2):
                        nc.tensor.matmul(
                            pc[:, m, :], wcx(k, m), xc[:, k, off:off + CH_],
                            start=False, stop=(k == 1),
                        )
                # throwaway identity matmuls pad the PE stream so its busy
                # streak (and therefore full clock) survives ACT pacing
                for f in range(2):
                    fill = psp.tile([128, 2, CH_], f32, tag="pp")
                    nc.tensor.matmul(
                        fill[:, :, :], eye, xc[:, :, off:off + CH_],
                        start=True, stop=True, skip_group_check=True,
                    )
                # stash preactivations for sweeps 2..K
                nc.scalar.activation(gx[:, :, s:s + CH_], pg[:], AF.Copy)
                nc.vector.tensor_scalar(
                    cx[:, :, s:s + CH_], pc[:], 0.0, None, ALU.add)
                nc.scalar.activation(
                    rub_t[:, 2:4, jsl], pg[:, 2:4, :], AF.Sigmoid)
                nc.scalar.activation(cb[:, :, co:co + CH_], pc[:], AF.Tanh)

            def gates_chunk(b, j, rub_t):
                """Gate preactivations + sigmoid for one chunk."""
                s = b * T + j * CH_
                first = (j == 0)
                hs = s if first else s - 1
                ncols = CH_ - 1 if first else CH_
                o0 = 1 if first else 0
                jsl = slice(j * CH_, (j + 1) * CH_)
                pg = psg.tile([128, 4, CH_], f32, tag="pg")
                # Gx injection: one 512-col identity matmul per bank
                nc.tensor.matmul(
                    pg[:, 0:2, :], eye, gx[:, 0:2, s:s + CH_],
                    start=True, stop=False)
                nc.tensor.matmul(
                    pg[:, 2:4, :], eye, gx[:, 2:4, s:s + CH_],
                    start=True, stop=False)
                for m in range(4):
                    for k in range(2):
                        nc.tensor.matmul(
                            pg[:, m, o0:CH_], wgh(k, m),
                            h[:, k, hs:hs + ncols],
                            start=False, stop=(k == 1),
                        )
                # r first: the rh product (and thus the candidate matmuls)
                # waits only on the r half; u feeds nothing until the scan
                nc.scalar.activation(
                    rub_t[:, 0:2, jsl], pg[:, 0:2, :], AF.Sigmoid)
                nc.scalar.activation(
                    rub_t[:, 2:4, jsl], pg[:, 2:4, :], AF.Sigmoid)

            def cand_pair(b, jp, rub_t, cb):
                """r*h, candidate matmuls and tanh for chunks 2jp, 2jp+1.
                Pairing the r*h products halves their DVE overhead and gives
                the in-order DVE queue ~2.4us of slack per pair for scans."""
                j0 = 2 * jp
                s = b * T + j0 * CH_
                first = (j0 == 0)
                hs = s if first else s - 1
                ncols = 2 * CH_ - 1 if first else 2 * CH_
                o0 = 1 if first else 0
                psl = slice(j0 * CH_, (j0 + 2) * CH_)
                rh_t = rhsc.tile([128, 2, 2 * CH_], bf16, tag="rh")
                nc.vector.tensor_mul(
                    rh_t[:, :, o0:2 * CH_],
                    rub_t[:, 0:2, psl][:, :, o0:2 * CH_],
                    h[:, :, hs:hs + ncols])
                for jj in range(2):
                    j = j0 + jj
                    oc = jj * CH_
                    oo = 1 if j == 0 else 0
                    pc = psc.tile([128, 2, CH_], f32, tag="pc")
                    nc.tensor.matmul(
                        pc[:, :, :], eye,
                        cx[:, :, (b * T + j * CH_):(b * T + (j + 1) * CH_)],
                        start=True, stop=False)
                    for m in range(2):
                        for k in range(2):
                            nc.tensor.matmul(
                                pc[:, m, oo:CH_], wch(k, m),
                                rh_t[:, k, oc + oo:oc + CH_],
                                start=False, stop=(k == 1),
                            )
                    nc.scalar.activation(
                        cb[:, :, j * CH_:(j + 1) * CH_], pc[:], AF.Tanh)

            def zbatch(rub_t, cb):
                """z = (u-1)*c over the whole block, overwriting the r half."""
                nc.vector.scalar_tensor_tensor(
                    rub_t[:, 0:2, :], rub_t[:, 2:4, :],
                    1.0, cb[:], ALU.subtract, ALU.mult)

            def scan(b, rub_t, kk, lo, hi, init):
                nc.vector.tensor_tensor_scan(
                    h[:, kk, b * T + lo:b * T + hi],
                    rub_t[:, 2 + kk, lo:hi], rub_t[:, kk, lo:hi],
                    init, ALU.mult, ALU.subtract)

            def project(b, jlo, jhi):
                for jj in range(jlo, jhi):
                    s = b * T + jj * CH_
                    if jj % OPB == 0:
                        ob = obp.tile([128, 2, OBLK], f32, tag="ob")
                        project.ob = ob
                    pp = psp.tile([128, 2, CH_], f32, tag="pp")
                    for mo in range(2):
                        for k in range(2):
                            nc.tensor.matmul(
                                pp[:, mo, :], wp(k, mo), h[:, k, s:s + CH_],
                                start=(mo == 0 and k == 0),
                                stop=(mo == 1 and k == 1),
                            )
                    oo = (jj % OPB) * CH_
                    # bp folded into the activation's per-partition bias
                    for mo in range(2):
                        nc.scalar.activation(
                            project.ob[:, mo, oo:oo + CH_], pp[:, mo, :],
                            AF.Sigmoid,
                            bias=w2[:, PK2_BP + mo:PK2_BP + mo + 1])
                    if jj % OPB == OPB - 1:
                        s0 = b * T + (jj - (OPB - 1)) * CH_
                        nc.sync.dma_start(
                            out=outT_d[:, :, s0:s0 + OBLK], in_=project.ob[:])

            # ---- block stream: sweep 1 (staging) then sweeps 2..K ----
            pending = []   # [(b, rub_t, do_proj)] scans awaiting emission

            def flush(stage):
                """Emit one full scan of the previous block (kk = stage).
                Positioned after pair 0 / pair 2 of the current block so the
                DVE always has a fresh rh pair banked ahead of each scan."""
                if not pending:
                    return
                pb, prub, dp = pending[0]
                scan(pb, prub, stage, 0, T, 0.0)
                if stage == 1:
                    if dp:
                        project(pb, 0, CPB_)
                    pending.pop(0)

            def xprefetch(b, j):
                # consume the group prefetched one XBLK ago and prefetch the
                # next so chunk 0 never waits on DMA
                xc = xprefetch.nxt if b + j > 0 else xc0
                s0 = b * T + j * CH_ + XBLK
                if s0 < BLOC * T:
                    nxt = xcp.tile([128, 2, XBLK], bf16, tag="xc")
                    for k in range(2):
                        nc.sync.dma_start(
                            out=nxt[:, k, :], in_=xT_d[k, :, s0:s0 + XBLK])
                    xprefetch.nxt = nxt
                return xc

            for kiter in range(K_):
                s1 = (kiter == 0)
                last = (kiter == K_ - 1)
                for b in range(BLOC):
                    rub_t = rubp.tile([128, 4, T], bf16, tag="ru")
                    cb = csc.tile([128, 2, T], bf16, tag="c")
                    if s1:
                        for j in range(CPB_):
                            if j % PBLK == 0:
                                xc = xprefetch(b, j)
                            sweep1_chunk(b, j, xc, rub_t, cb)
                            if j == 2:
                                flush(0)
                            elif j == 6:
                                flush(1)
                    else:
                        for jp in range(CPB_ // 2):
                            gates_chunk(b, 2 * jp, rub_t)
                            gates_chunk(b, 2 * jp + 1, rub_t)
                            cand_pair(b, jp, rub_t, cb)
                            if jp == 1:
                                flush(0)
                            elif jp == 3:
                                flush(1)
                    zbatch(rub_t, cb)
                    pending.append((b, rub_t, last))

            # drain the final block: quarter scans, projection right behind
            fb, frub, _ = pending.pop(0)
            qt = T // 4
            for q in range(4):
                lo = q * qt
                for kk in range(2):
                    init = (0.0 if q == 0
                            else h[:, kk, fb * T + lo - 1:fb * T + lo])
                    scan(fb, frub, kk, lo, lo + qt, init)
                project(fb, q * (CPB_ // 4), (q + 1) * (CPB_ // 4))

    nc.finalize()
    return nc


def _get_nc(K_, CH_):
    key = (K_, CH_)
    if key not in _cache:
        _cache[key] = _build(K_, CH_)
    return _cache[key]


def _pack_weights(Wg, bg, Wc, bc, Wp, bp):
    import ml_dtypes

    bf16 = ml_dtypes.bfloat16
    w1 = np.zeros((128, PKW1), dtype=bf16)
    w2 = np.zeros((128, PKW2), dtype=bf16)

    def put(w, off, a):  # a: [2, 128, X] -> cols [off : off + 2X]
        X = a.shape[2]
        for k in range(2):
            w[:, off + k * X:off + (k + 1) * X] = a[k].astype(bf16)

    put(w1, PK1_WGX, Wg[:256].reshape(2, 128, 512))
    put(w1, PK1_WCX, Wc[:256].reshape(2, 128, 256))
    w1[:, PK1_EYE:PK1_EYE + 128] = np.eye(128, dtype=np.float32).astype(bf16)
    w1[0:2, PK1_BG01:PK1_BG01 + 128] = bg[:256].reshape(2, 128).astype(bf16)
    w1[0:2, PK1_BG23:PK1_BG23 + 128] = bg[256:].reshape(2, 128).astype(bf16)
    w1[0:2, PK1_BC:PK1_BC + 128] = bc.reshape(2, 128).astype(bf16)
    for r in range(2):
        w1[r, PK1_MASK + r * CH:PK1_MASK + (r + 1) * CH] = bf16(1.0)
    put(w2, PK2_WGH, Wg[256:].reshape(2, 128, 512))
    put(w2, PK2_WCH, Wc[256:].reshape(2, 128, 256))
    put(w2, PK2_WP, Wp.reshape(2, 128, 256))
    w2[:, PK2_BP:PK2_BP + 2] = bp.reshape(2, 128).T.astype(bf16)
    return w1, w2


def run_gru(x, Wg, bg, Wc, bc, Wp, bp, K_=None, CH_=None, trace=False):
    from concourse.bass_utils import run_bass_kernel_spmd
    import ml_dtypes

    K_ = K_ or K
    CH_ = CH_ or CH
    x = np.asarray(x, dtype=np.float32)
    nc = _get_nc(K_, CH_)
    w1, w2 = _pack_weights(Wg, bg, Wc, bc, Wp, bp)
    in_maps = []
    for core in range(NCORES):
        x_core = x[core * BLOC:(core + 1) * BLOC]
        xT = np.ascontiguousarray(
            x_core.transpose(2, 0, 1).reshape(2, 128, N).astype(
                ml_dtypes.bfloat16))
        in_maps.append({"xT": xT, "wpk1": w1, "wpk2": w2})
    res = run_bass_kernel_spmd(nc, in_maps, list(range(NCORES)), trace=trace)
    outs = []
    for core in range(NCORES):
        oT = res.results[core]["outT"]  # [128, 2, N]
        o = (oT.reshape(128, 2, BLOC, T)
             .transpose(2, 3, 1, 0).reshape(BLOC, T, OUT))
        outs.append(o)
    full = np.concatenate(outs, axis=0).astype(np.float32)
    return full, res


def kernel(x, Wg, bg, Wc, bc, Wp, bp):
    out, _ = run_gru(
        np.asarray(x), np.asarray(Wg), np.asarray(bg), np.asarray(Wc),
        np.asarray(bc), np.asarray(Wp), np.asarray(bp),
    )
    return out


# revision 56
# speedup vs baseline: 1.0318x; 1.0318x over previous
"""Trainium2 Bass kernel for a TF-style GRU + sigmoid projection.

Reference computation (B=32, T=2048, D=H=OUT=256):
    ru  = sigmoid([x_t, h] @ Wg + bg);  r, u = split(ru)
    c   = tanh([x_t, r*h] @ Wc + bc)
    h'  = u*h + (1-u)*c
    out = sigmoid(H @ Wp + bp)          # H = all h_t

Strategy: data-parallel over batch (8 cores x 4 sequences), and
parallel-in-time inside each core via fixed-point (quasi-DEER) sweeps:

    sweep k:  for ALL t in parallel (big matmuls, full engine occupancy):
                  pr,pu = Gx_t + Wgh @ h^{k-1}_{t-1};  r,u = sigmoid
                  c     = tanh(Cx_t + Wch @ (r * h^{k-1}_{t-1}))
                  z     = (u-1)*c            # -(1-u)*c
              then one hardware prefix scan per (k-tile, seq):
                  h^k_t = u_t * h^k_{t-1} - z_t     (tensor_tensor_scan)

The scan makes the u-memory chain exact every sweep; only the gate/candidate
coupling iterates, contracting ~0.37x per sweep.  K=3 sweeps reach ~5e-3
rel L2 (gate is 2e-2).  Sweep 1 (h=0) doubles as the Gx/Cx staging pass.

Scheduling notes:
  - The two 4.4us serial scans per (sweep, seq) are DEFERRED into the next
    seq-block's instruction stream (after chunks 2 and 6) so the in-order
    DVE queue never head-of-line-blocks the rh products the PE is waiting
    on.  Projection of block b runs right after b's second deferred scan.
  - z = (u-1)*c is batched over ZBLK columns to amortize DVE overhead.
  - Sweep 1 issues two throwaway 512-col identity matmuls per chunk to keep
    the otherwise ACT-paced PE stream dense enough to hold full clock
    (TRN2 PE drops to 1.2GHz when its busy streak breaks).

Everything on chip is hidden-major: [128 partitions = half the hidden dim,
2 k-tiles, cols] with col = seq*2048 + t (t fastest, so the scan can run
along the free dimension per sequence).
"""

import numpy as np

B, T, D = 32, 2048, 256
H, OUT = 256, 256
NCORES = 8
BLOC = B // NCORES      # 4 sequences per core
N = T * BLOC            # 8192 cols, col = b*T + t
CH = 256                # cols per psum chunk
CPB = T // CH           # chunks per sequence
XBLK = 1024             # x-stream DMA block
OBLK = 256              # output DMA block
ZBLK = 1024             # cols per batched z (stt) op
K = 3                   # fixed-point sweeps

# packed-weight column offsets (bf16).  wpk1 = sweep-1 set, wpk2 = the rest.
PK1_WGX = 0       # [2k x 512]
PK1_WCX = 1024    # [2k x 256]
PK1_EYE = 1536    # [128]
PK1_BG01 = 1664   # rows 0-1: bg[m*128+p] for m=0,1 (transposed bias)
PK1_BG23 = 1792   # rows 0-1: bg[(2+m)*128+p]
PK1_BC = 1920     # rows 0-1: bc[m*128+p]
PK1_MASK = 2048   # rows 0-1: one-hot [2 x 2*CH] (mask[r, m*CH+cc] = r==m)
PKW1 = 2560
PK2_WGH = 0       # [2k x 512]
PK2_WCH = 1024    # [2k x 256]
PK2_WP = 1536     # [2k x 256]
PK2_BP = 2048     # cols 2048+mo: bp[mo*128+p] as [128,1] columns
PKW2 = 2050

_cache = {}


def _build(K_, CH_):
    import concourse.bacc as bacc
    import concourse.mybir as mybir
    from concourse.tile import TileContext

    f32 = mybir.dt.float32
    bf16 = mybir.dt.bfloat16
    AF = mybir.ActivationFunctionType
    ALU = mybir.AluOpType

    CPB_ = T // CH_
    PBLK = XBLK // CH_   # chunks per x DMA block
    OPB = OBLK // CH_    # chunks per out DMA block
    ZPB = ZBLK // CH_    # chunks per batched z op

    nc = bacc.Bacc("TRN2", target_bir_lowering=False, debug=False)

    xT_d = nc.declare_dram_parameter("xT", [2, 128, N], bf16, isOutput=False)
    wpk1_d = nc.declare_dram_parameter("wpk1", [128, PKW1], bf16,
                                       isOutput=False)
    wpk2_d = nc.declare_dram_parameter("wpk2", [128, PKW2], bf16,
                                       isOutput=False)
    outT_d = nc.declare_dram_parameter("outT", [128, 2, N], f32, isOutput=True)

    with TileContext(nc) as tc:
        with (
            tc.tile_pool(name="const", bufs=1) as const,
            tc.tile_pool(name="xc", bufs=2) as xcp,
            tc.tile_pool(name="csc", bufs=2) as csc,
            tc.tile_pool(name="rhsc", bufs=2) as rhsc,
            tc.tile_pool(name="rub", bufs=2) as rubp,
            tc.tile_pool(name="ob", bufs=2) as obp,
            tc.tile_pool(name="psg", bufs=2, space="PSUM") as psg,
            tc.tile_pool(name="psc", bufs=2, space="PSUM") as psc,
            tc.tile_pool(name="psp", bufs=2, space="PSUM") as psp,
        ):
            gx = const.tile([128, 4, N], bf16)   # Gx+bg, m = [r0,r1,u0,u1]
            cx = const.tile([128, 2, N], bf16)   # Cx+bc
            h = const.tile([128, 2, N], bf16)
            w1 = const.tile([128, PKW1], bf16)
            w2 = const.tile([128, PKW2], bf16)

            # boot DMAs on separate engine queues so the transfers overlap;
            # the small eye/bias/mask range lands first so the first chunk's
            # bias matmuls can issue while wgx/wcx stream in
            xc0 = xcp.tile([128, 2, XBLK], bf16, tag="xc")
            nc.sync.dma_start(out=w1[:, PK1_EYE:], in_=wpk1_d[:, PK1_EYE:])
            for k in range(2):
                nc.scalar.dma_start(out=xc0[:, k, :], in_=xT_d[k, :, 0:XBLK])
            nc.sync.dma_start(out=w1[:, :PK1_EYE], in_=wpk1_d[:, :PK1_EYE])
            nc.gpsimd.dma_start(out=w2[:], in_=wpk2_d[:])
            # one-hot mask moving operand: broadcasts a transposed bias row
            # into both m-tiles of a psum tile with ONE 2-partition matmul
            # (1-partition `ones` moving operands cost a PE pipeline drain)
            mask2 = w1[0:2, PK1_MASK:PK1_MASK + 2 * CH_]

            def wgx(k, m):
                return w1[:, PK1_WGX + k * 512 + m * 128:
                          PK1_WGX + k * 512 + (m + 1) * 128]

            def wcx(k, m):
                return w1[:, PK1_WCX + k * 256 + m * 128:
                          PK1_WCX + k * 256 + (m + 1) * 128]

            def wgh(k, m):
                return w2[:, PK2_WGH + k * 512 + m * 128:
                          PK2_WGH + k * 512 + (m + 1) * 128]

            def wch(k, m):
                return w2[:, PK2_WCH + k * 256 + m * 128:
                          PK2_WCH + k * 256 + (m + 1) * 128]

            def wp(k, m):
                return w2[:, PK2_WP + k * 256 + m * 128:
                          PK2_WP + k * 256 + (m + 1) * 128]

            eye = w1[:, PK1_EYE:PK1_EYE + 128]

            def sweep1_chunk(b, j, xc, rub_t, cb):
                """pg/pc = x-part + bias; store Gx/Cx; u, c for the scan."""
                s = b * T + j * CH_
                off = (j % PBLK) * CH_
                co = j * CH_
                jsl = slice(j * CH_, (j + 1) * CH_)
                pg = psg.tile([128, 4, CH_], f32, tag="pg")
                pc = psc.tile([128, 2, CH_], f32, tag="pc")
                # clustered transposed-bias matmuls first (start=True resets
                # each bank); all three share the mask2 moving operand so the
                # PE pays at most one moving-partition-count transition
                nc.tensor.matmul(
                    pg[:, 0:2, :], w1[0:2, PK1_BG01:PK1_BG01 + 128],
                    mask2, start=True, stop=False)
                nc.tensor.matmul(
                    pg[:, 2:4, :], w1[0:2, PK1_BG23:PK1_BG23 + 128],
                    mask2, start=True, stop=False)
                nc.tensor.matmul(
                    pc[:, :, :], w1[0:2, PK1_BC:PK1_BC + 128],
                    mask2, start=True, stop=False)
                for m in range(4):
                    for k in range(2):
                        nc.tensor.matmul(
                            pg[:, m, :], wgx(k, m), xc[:, k, off:off + CH_],
                            start=False, stop=(k == 1),
                        )
                for m in range(2):
                    for k in range(2):
                        nc.tensor.matmul(
                            pc[:, m, :], wcx(k, m), xc[:, k, off:off + CH_],
                            start=False, stop=(k == 1),
                        )
                # throwaway identity matmuls pad the PE stream so its busy
                # streak (and therefore full clock) survives ACT pacing
                for f in range(2):
                    fill = psp.tile([128, 2, CH_], f32, tag="pp")
                    nc.tensor.matmul(
                        fill[:, :, :], eye, xc[:, :, off:off + CH_],
                        start=True, stop=True, skip_group_check=True,
                    )
                # stash preactivations for sweeps 2..K
                nc.scalar.activation(gx[:, :, s:s + CH_], pg[:], AF.Copy)
                nc.vector.tensor_scalar(
                    cx[:, :, s:s + CH_], pc[:], 0.0, None, ALU.add)
                nc.scalar.activation(
                    rub_t[:, 2:4, jsl], pg[:, 2:4, :], AF.Sigmoid)
                nc.scalar.activation(cb[:, :, co:co + CH_], pc[:], AF.Tanh)

            def gates_chunk(b, j, rub_t):
                """Gate preactivations + sigmoid for one chunk."""
                s = b * T + j * CH_
                first = (j == 0)
                hs = s if first else s - 1
                ncols = CH_ - 1 if first else CH_
                o0 = 1 if first else 0
                jsl = slice(j * CH_, (j + 1) * CH_)
                pg = psg.tile([128, 4, CH_], f32, tag="pg")
                # Gx injection: one 512-col identity matmul per bank
                nc.tensor.matmul(
                    pg[:, 0:2, :], eye, gx[:, 0:2, s:s + CH_],
                    start=True, stop=False)
                nc.tensor.matmul(
                    pg[:, 2:4, :], eye, gx[:, 2:4, s:s + CH_],
                    start=True, stop=False)
                for m in range(4):
                    for k in range(2):
                        nc.tensor.matmul(
                            pg[:, m, o0:CH_], wgh(k, m),
                            h[:, k, hs:hs + ncols],
                            start=False, stop=(k == 1),
                        )
                nc.scalar.activation(rub_t[:, :, jsl], pg[:], AF.Sigmoid)

            def cand_pair(b, jp, rub_t, cb):
                """r*h, candidate matmuls and tanh for chunks 2jp, 2jp+1.
                Pairing the r*h products halves their DVE overhead and gives
                the in-order DVE queue ~2.4us of slack per pair for scans."""
                j0 = 2 * jp
                s = b * T + j0 * CH_
                first = (j0 == 0)
                hs = s if first else s - 1
                ncols = 2 * CH_ - 1 if first else 2 * CH_
                o0 = 1 if first else 0
                psl = slice(j0 * CH_, (j0 + 2) * CH_)
                rh_t = rhsc.tile([128, 2, 2 * CH_], bf16, tag="rh")
                nc.vector.tensor_mul(
                    rh_t[:, :, o0:2 * CH_],
                    rub_t[:, 0:2, psl][:, :, o0:2 * CH_],
                    h[:, :, hs:hs + ncols])
                for jj in range(2):
                    j = j0 + jj
                    oc = jj * CH_
                    oo = 1 if j == 0 else 0
                    pc = psc.tile([128, 2, CH_], f32, tag="pc")
                    nc.tensor.matmul(
                        pc[:, :, :], eye,
                        cx[:, :, (b * T + j * CH_):(b * T + (j + 1) * CH_)],
                        start=True, stop=False)
                    for m in range(2):
                        for k in range(2):
                            nc.tensor.matmul(
                                pc[:, m, oo:CH_], wch(k, m),
                                rh_t[:, k, oc + oo:oc + CH_],
                                start=False, stop=(k == 1),
                            )
                    nc.scalar.activation(
                        cb[:, :, j * CH_:(j + 1) * CH_], pc[:], AF.Tanh)

            def zbatch(rub_t, cb):
                """z = (u-1)*c over the whole block, overwriting the r half."""
                nc.vector.scalar_tensor_tensor(
                    rub_t[:, 0:2, :], rub_t[:, 2:4, :],
                    1.0, cb[:], ALU.subtract, ALU.mult)

            def scan(b, rub_t, kk, lo, hi, init):
                nc.vector.tensor_tensor_scan(
                    h[:, kk, b * T + lo:b * T + hi],
                    rub_t[:, 2 + kk, lo:hi], rub_t[:, kk, lo:hi],
                    init, ALU.mult, ALU.subtract)

            def project(b, jlo, jhi):
                for jj in range(jlo, jhi):
                    s = b * T + jj * CH_
                    if jj % OPB == 0:
                        ob = obp.tile([128, 2, OBLK], f32, tag="ob")
                        project.ob = ob
                    pp = psp.tile([128, 2, CH_], f32, tag="pp")
                    for mo in range(2):
                        for k in range(2):
                            nc.tensor.matmul(
                                pp[:, mo, :], wp(k, mo), h[:, k, s:s + CH_],
                                start=(mo == 0 and k == 0),
                                stop=(mo == 1 and k == 1),
                            )
                    oo = (jj % OPB) * CH_
                    # bp folded into the activation's per-partition bias
                    for mo in range(2):
                        nc.scalar.activation(
                            project.ob[:, mo, oo:oo + CH_], pp[:, mo, :],
                            AF.Sigmoid,
                            bias=w2[:, PK2_BP + mo:PK2_BP + mo + 1])
                    if jj % OPB == OPB - 1:
                        s0 = b * T + (jj - (OPB - 1)) * CH_
                        nc.sync.dma_start(
                            out=outT_d[:, :, s0:s0 + OBLK], in_=project.ob[:])

            # ---- block stream: sweep 1 (staging) then sweeps 2..K ----
            pending = []   # [(b, rub_t, do_proj)] scans awaiting emission

            def flush(stage):
                """Emit one full scan of the previous block (kk = stage).
                Positioned after pair 0 / pair 2 of the current block so the
                DVE always has a fresh rh pair banked ahead of each scan."""
                if not pending:
                    return
                pb, prub, dp = pending[0]
                scan(pb, prub, stage, 0, T, 0.0)
                if stage == 1:
                    if dp:
                        project(pb, 0, CPB_)
                    pending.pop(0)

            def xprefetch(b, j):
                # consume the group prefetched one XBLK ago and prefetch the
                # next so chunk 0 never waits on DMA
                xc = xprefetch.nxt if b + j > 0 else xc0
                s0 = b * T + j * CH_ + XBLK
                if s0 < BLOC * T:
                    nxt = xcp.tile([128, 2, XBLK], bf16, tag="xc")
                    for k in range(2):
                        nc.sync.dma_start(
                            out=nxt[:, k, :], in_=xT_d[k, :, s0:s0 + XBLK])
                    xprefetch.nxt = nxt
                return xc

            for kiter in range(K_):
                s1 = (kiter == 0)
                last = (kiter == K_ - 1)
                for b in range(BLOC):
                    rub_t = rubp.tile([128, 4, T], bf16, tag="ru")
                    cb = csc.tile([128, 2, T], bf16, tag="c")
                    if s1:
                        for j in range(CPB_):
                            if j % PBLK == 0:
                                xc = xprefetch(b, j)
                            sweep1_chunk(b, j, xc, rub_t, cb)
                            if j == 2:
                                flush(0)
                            elif j == 6:
                                flush(1)
                    else:
                        for jp in range(CPB_ // 2):
                            gates_chunk(b, 2 * jp, rub_t)
                            gates_chunk(b, 2 * jp + 1, rub_t)
                            cand_pair(b, jp, rub_t, cb)
                            if jp == 1:
                                flush(0)
                            elif jp == 3:
                                flush(1)
                    zbatch(rub_t, cb)
                    pending.append((b, rub_t, last))

            # drain the final block: quarter scans, projection right behind
            fb, frub, _ = pending.pop(0)
            qt = T // 4
            for q in range(4):
                lo = q * qt
                for kk in range(2):
                    init = (0.0 if q == 0
                            else h[:, kk, fb * T + lo - 1:fb * T + lo])
                    scan(fb, frub, kk, lo, lo + qt, init)
                project(fb, q * (CPB_ // 4), (q + 1) * (CPB_ // 4))

    nc.finalize()
    return nc


def _get_nc(K_, CH_):
    key = (K_, CH_)
    if key not in _cache:
        _cache[key] = _build(K_, CH_)
    return _cache[key]


def _pack_weights(Wg, bg, Wc, bc, Wp, bp):
    import ml_dtypes

    bf16 = ml_dtypes.bfloat16
    w1 = np.zeros((128, PKW1), dtype=bf16)
    w2 = np.zeros((128, PKW2), dtype=bf16)

    def put(w, off, a):  # a: [2, 128, X] -> cols [off : off + 2X]
        X = a.shape[2]
        for k in range(2):
            w[:, off + k * X:off + (k + 1) * X] = a[k].astype(bf16)

    put(w1, PK1_WGX, Wg[:256].reshape(2, 128, 512))
    put(w1, PK1_WCX, Wc[:256].reshape(2, 128, 256))
    w1[:, PK1_EYE:PK1_EYE + 128] = np.eye(128, dtype=np.float32).astype(bf16)
    w1[0:2, PK1_BG01:PK1_BG01 + 128] = bg[:256].reshape(2, 128).astype(bf16)
    w1[0:2, PK1_BG23:PK1_BG23 + 128] = bg[256:].reshape(2, 128).astype(bf16)
    w1[0:2, PK1_BC:PK1_BC + 128] = bc.reshape(2, 128).astype(bf16)
    for r in range(2):
        w1[r, PK1_MASK + r * CH:PK1_MASK + (r + 1) * CH] = bf16(1.0)
    put(w2, PK2_WGH, Wg[256:].reshape(2, 128, 512))
    put(w2, PK2_WCH, Wc[256:].reshape(2, 128, 256))
    put(w2, PK2_WP, Wp.reshape(2, 128, 256))
    w2[:, PK2_BP:PK2_BP + 2] = bp.reshape(2, 128).T.astype(bf16)
    return w1, w2


def run_gru(x, Wg, bg, Wc, bc, Wp, bp, K_=None, CH_=None, trace=False):
    from concourse.bass_utils import run_bass_kernel_spmd
    import ml_dtypes

    K_ = K_ or K
    CH_ = CH_ or CH
    x = np.asarray(x, dtype=np.float32)
    nc = _get_nc(K_, CH_)
    w1, w2 = _pack_weights(Wg, bg, Wc, bc, Wp, bp)
    in_maps = []
    for core in range(NCORES):
        x_core = x[core * BLOC:(core + 1) * BLOC]
        xT = np.ascontiguousarray(
            x_core.transpose(2, 0, 1).reshape(2, 128, N).astype(
                ml_dtypes.bfloat16))
        in_maps.append({"xT": xT, "wpk1": w1, "wpk2": w2})
    res = run_bass_kernel_spmd(nc, in_maps, list(range(NCORES)), trace=trace)
    outs = []
    for core in range(NCORES):
        oT = res.results[core]["outT"]  # [128, 2, N]
        o = (oT.reshape(128, 2, BLOC, T)
             .transpose(2, 3, 1, 0).reshape(BLOC, T, OUT))
        outs.append(o)
    full = np.concatenate(outs, axis=0).astype(np.float32)
    return full, res


def kernel(x, Wg, bg, Wc, bc, Wp, bp):
    out, _ = run_gru(
        np.asarray(x), np.asarray(Wg), np.asarray(bg), np.asarray(Wc),
        np.asarray(bc), np.asarray(Wp), np.asarray(bp),
    )
    return out


# revision 57
# speedup vs baseline: 1.0502x; 1.0178x over previous
"""Trainium2 Bass kernel for a TF-style GRU + sigmoid projection.

Reference computation (B=32, T=2048, D=H=OUT=256):
    ru  = sigmoid([x_t, h] @ Wg + bg);  r, u = split(ru)
    c   = tanh([x_t, r*h] @ Wc + bc)
    h'  = u*h + (1-u)*c
    out = sigmoid(H @ Wp + bp)          # H = all h_t

Strategy: data-parallel over batch (8 cores x 4 sequences), and
parallel-in-time inside each core via fixed-point (quasi-DEER) sweeps:

    sweep k:  for ALL t in parallel (big matmuls, full engine occupancy):
                  pr,pu = Gx_t + Wgh @ h^{k-1}_{t-1};  r,u = sigmoid
                  c     = tanh(Cx_t + Wch @ (r * h^{k-1}_{t-1}))
                  z     = (u-1)*c            # -(1-u)*c
              then one hardware prefix scan per (k-tile, seq):
                  h^k_t = u_t * h^k_{t-1} - z_t     (tensor_tensor_scan)

The scan makes the u-memory chain exact every sweep; only the gate/candidate
coupling iterates, contracting ~0.37x per sweep.  K=3 sweeps reach ~5e-3
rel L2 (gate is 2e-2).  Sweep 1 (h=0) doubles as the Gx/Cx staging pass.

Scheduling notes:
  - The two 4.4us serial scans per (sweep, seq) are DEFERRED into the next
    seq-block's instruction stream (after chunks 2 and 6) so the in-order
    DVE queue never head-of-line-blocks the rh products the PE is waiting
    on.  Projection of block b runs right after b's second deferred scan.
  - z = (u-1)*c is batched over ZBLK columns to amortize DVE overhead.
  - Sweep 1 issues two throwaway 512-col identity matmuls per chunk to keep
    the otherwise ACT-paced PE stream dense enough to hold full clock
    (TRN2 PE drops to 1.2GHz when its busy streak breaks).

Everything on chip is hidden-major: [128 partitions = half the hidden dim,
2 k-tiles, cols] with col = seq*2048 + t (t fastest, so the scan can run
along the free dimension per sequence).
"""

import numpy as np

B, T, D = 32, 2048, 256
H, OUT = 256, 256
NCORES = 8
BLOC = B // NCORES      # 4 sequences per core
N = T * BLOC            # 8192 cols, col = b*T + t
CH = 256                # cols per psum chunk
CPB = T // CH           # chunks per sequence
XBLK = 1024             # x-stream DMA block
OBLK = 256              # output DMA block
ZBLK = 1024             # cols per batched z (stt) op
K = 3                   # fixed-point sweeps

# packed-weight column offsets (bf16).  wpk1 = sweep-1 set, wpk2 = the rest.
PK1_WGX = 0       # [2k x 512]
PK1_WCX = 1024    # [2k x 256]
PK1_EYE = 1536    # [128]
PK1_BG01 = 1664   # rows 0-1: bg[m*128+p] for m=0,1 (transposed bias)
PK1_BG23 = 1792   # rows 0-1: bg[(2+m)*128+p]
PK1_BC = 1920     # rows 0-1: bc[m*128+p]
PK1_MASK = 2048   # rows 0-1: one-hot [2 x 2*CH] (mask[r, m*CH+cc] = r==m)
PKW1 = 2560
PK2_WGH = 0       # [2k x 512]
PK2_WCH = 1024    # [2k x 256]
PK2_WP = 1536     # [2k x 256]
PK2_BP = 2048     # cols 2048+mo: bp[mo*128+p] as [128,1] columns
PKW2 = 2050

_cache = {}


def _build(K_, CH_):
    import concourse.bacc as bacc
    import concourse.mybir as mybir
    from concourse.tile import TileContext

    f32 = mybir.dt.float32
    bf16 = mybir.dt.bfloat16
    AF = mybir.ActivationFunctionType
    ALU = mybir.AluOpType

    CPB_ = T // CH_
    PBLK = XBLK // CH_   # chunks per x DMA block
    OPB = OBLK // CH_    # chunks per out DMA block
    ZPB = ZBLK // CH_    # chunks per batched z op

    nc = bacc.Bacc("TRN2", target_bir_lowering=False, debug=False)

    xT_d = nc.declare_dram_parameter("xT", [2, 128, N], bf16, isOutput=False)
    wpk1_d = nc.declare_dram_parameter("wpk1", [128, PKW1], bf16,
                                       isOutput=False)
    wpk2_d = nc.declare_dram_parameter("wpk2", [128, PKW2], bf16,
                                       isOutput=False)
    outT_d = nc.declare_dram_parameter("outT", [128, 2, N], f32, isOutput=True)

    with TileContext(nc) as tc:
        with (
            tc.tile_pool(name="const", bufs=1) as const,
            tc.tile_pool(name="xc", bufs=2) as xcp,
            tc.tile_pool(name="csc", bufs=2) as csc,
            tc.tile_pool(name="rhsc", bufs=2) as rhsc,
            tc.tile_pool(name="rub", bufs=2) as rubp,
            tc.tile_pool(name="ob", bufs=2) as obp,
            tc.tile_pool(name="psg", bufs=2, space="PSUM") as psg,
            tc.tile_pool(name="psc", bufs=2, space="PSUM") as psc,
            tc.tile_pool(name="psp", bufs=2, space="PSUM") as psp,
        ):
            gx = const.tile([128, 4, N], bf16)   # Gx+bg, m = [r0,r1,u0,u1]
            cx = const.tile([128, 2, N], bf16)   # Cx+bc
            h = const.tile([128, 2, N], bf16)
            w1 = const.tile([128, PKW1], bf16)
            w2 = const.tile([128, PKW2], bf16)

            # boot DMAs on separate engine queues so the transfers overlap;
            # the small eye/bias/mask range lands first so the first chunk's
            # bias matmuls can issue while wgx/wcx stream in
            xc0 = xcp.tile([128, 2, XBLK], bf16, tag="xc")
            nc.sync.dma_start(out=w1[:, PK1_EYE:], in_=wpk1_d[:, PK1_EYE:])
            for k in range(2):
                nc.scalar.dma_start(out=xc0[:, k, :], in_=xT_d[k, :, 0:XBLK])
            nc.sync.dma_start(out=w1[:, :PK1_EYE], in_=wpk1_d[:, :PK1_EYE])
            nc.gpsimd.dma_start(out=w2[:], in_=wpk2_d[:])
            # one-hot mask moving operand: broadcasts a transposed bias row
            # into both m-tiles of a psum tile with ONE 2-partition matmul
            # (1-partition `ones` moving operands cost a PE pipeline drain)
            mask2 = w1[0:2, PK1_MASK:PK1_MASK + 2 * CH_]

            def wgx(k, m):
                return w1[:, PK1_WGX + k * 512 + m * 128:
                          PK1_WGX + k * 512 + (m + 1) * 128]

            def wcx(k, m):
                return w1[:, PK1_WCX + k * 256 + m * 128:
                          PK1_WCX + k * 256 + (m + 1) * 128]

            def wgh(k, m):
                return w2[:, PK2_WGH + k * 512 + m * 128:
                          PK2_WGH + k * 512 + (m + 1) * 128]

            def wch(k, m):
                return w2[:, PK2_WCH + k * 256 + m * 128:
                          PK2_WCH + k * 256 + (m + 1) * 128]

            def wp(k, m):
                return w2[:, PK2_WP + k * 256 + m * 128:
                          PK2_WP + k * 256 + (m + 1) * 128]

            eye = w1[:, PK1_EYE:PK1_EYE + 128]

            def sweep1_chunk(b, j, xc, rub_t, cb):
                """pg/pc = x-part + bias; store Gx/Cx; u, c for the scan."""
                s = b * T + j * CH_
                off = (j % PBLK) * CH_
                co = j * CH_
                jsl = slice(j * CH_, (j + 1) * CH_)
                pg = psg.tile([128, 4, CH_], f32, tag="pg")
                pc = psc.tile([128, 2, CH_], f32, tag="pc")
                # clustered transposed-bias matmuls first (start=True resets
                # each bank); all three share the mask2 moving operand so the
                # PE pays at most one moving-partition-count transition
                nc.tensor.matmul(
                    pg[:, 0:2, :], w1[0:2, PK1_BG01:PK1_BG01 + 128],
                    mask2, start=True, stop=False)
                nc.tensor.matmul(
                    pg[:, 2:4, :], w1[0:2, PK1_BG23:PK1_BG23 + 128],
                    mask2, start=True, stop=False)
                nc.tensor.matmul(
                    pc[:, :, :], w1[0:2, PK1_BC:PK1_BC + 128],
                    mask2, start=True, stop=False)
                for m in range(4):
                    for k in range(2):
                        nc.tensor.matmul(
                            pg[:, m, :], wgx(k, m), xc[:, k, off:off + CH_],
                            start=False, stop=(k == 1),
                        )
                for m in range(2):
                    for k in range(2):
                        nc.tensor.matmul(
                            pc[:, m, :], wcx(k, m), xc[:, k, off:off + CH_],
                            start=False, stop=(k == 1),
                        )
                # throwaway identity matmuls pad the PE stream so its busy
                # streak (and therefore full clock) survives ACT pacing
                for f in range(2):
                    fill = psp.tile([128, 2, CH_], f32, tag="pp")
                    nc.tensor.matmul(
                        fill[:, :, :], eye, xc[:, :, off:off + CH_],
                        start=True, stop=True, skip_group_check=True,
                    )
                # stash preactivations for sweeps 2..K
                nc.scalar.activation(gx[:, :, s:s + CH_], pg[:], AF.Copy)
                nc.vector.tensor_scalar(
                    cx[:, :, s:s + CH_], pc[:], 0.0, None, ALU.add)
                nc.scalar.activation(
                    rub_t[:, 2:4, jsl], pg[:, 2:4, :], AF.Sigmoid)
                nc.scalar.activation(cb[:, :, co:co + CH_], pc[:], AF.Tanh)

            def gates_chunk(b, j, rub_t):
                """Gate preactivations + sigmoid for one chunk."""
                s = b * T + j * CH_
                first = (j == 0)
                hs = s if first else s - 1
                ncols = CH_ - 1 if first else CH_
                o0 = 1 if first else 0
                jsl = slice(j * CH_, (j + 1) * CH_)
                pg = psg.tile([128, 4, CH_], f32, tag="pg")
                # Gx injection: one 512-col identity matmul per bank
                nc.tensor.matmul(
                    pg[:, 0:2, :], eye, gx[:, 0:2, s:s + CH_],
                    start=True, stop=False)
                nc.tensor.matmul(
                    pg[:, 2:4, :], eye, gx[:, 2:4, s:s + CH_],
                    start=True, stop=False)
                for m in range(4):
                    for k in range(2):
                        nc.tensor.matmul(
                            pg[:, m, o0:CH_], wgh(k, m),
                            h[:, k, hs:hs + ncols],
                            start=False, stop=(k == 1),
                        )
                nc.scalar.activation(rub_t[:, :, jsl], pg[:], AF.Sigmoid)

            def cand_pair(b, jp, rub_t, cb):
                """r*h, candidate matmuls and tanh for chunks 2jp, 2jp+1.
                Pairing the r*h products halves their DVE overhead and gives
                the in-order DVE queue ~2.4us of slack per pair for scans."""
                j0 = 2 * jp
                s = b * T + j0 * CH_
                first = (j0 == 0)
                hs = s if first else s - 1
                ncols = 2 * CH_ - 1 if first else 2 * CH_
                o0 = 1 if first else 0
                psl = slice(j0 * CH_, (j0 + 2) * CH_)
                rh_t = rhsc.tile([128, 2, 2 * CH_], bf16, tag="rh")
                nc.vector.tensor_mul(
                    rh_t[:, :, o0:2 * CH_],
                    rub_t[:, 0:2, psl][:, :, o0:2 * CH_],
                    h[:, :, hs:hs + ncols])
                for jj in range(2):
                    j = j0 + jj
                    oc = jj * CH_
                    oo = 1 if j == 0 else 0
                    pc = psc.tile([128, 2, CH_], f32, tag="pc")
                    nc.tensor.matmul(
                        pc[:, :, :], eye,
                        cx[:, :, (b * T + j * CH_):(b * T + (j + 1) * CH_)],
                        start=True, stop=False)
                    for m in range(2):
                        for k in range(2):
                            nc.tensor.matmul(
                                pc[:, m, oo:CH_], wch(k, m),
                                rh_t[:, k, oc + oo:oc + CH_],
                                start=False, stop=(k == 1),
                            )
                    nc.scalar.activation(
                        cb[:, :, j * CH_:(j + 1) * CH_], pc[:], AF.Tanh)

            def zbatch(rub_t, cb):
                """z = (u-1)*c over the whole block, overwriting the r half."""
                nc.vector.scalar_tensor_tensor(
                    rub_t[:, 0:2, :], rub_t[:, 2:4, :],
                    1.0, cb[:], ALU.subtract, ALU.mult)

            def scan(b, rub_t, kk, lo, hi, init):
                nc.vector.tensor_tensor_scan(
                    h[:, kk, b * T + lo:b * T + hi],
                    rub_t[:, 2 + kk, lo:hi], rub_t[:, kk, lo:hi],
                    init, ALU.mult, ALU.subtract)

            def project(b, jlo, jhi):
                for jj in range(jlo, jhi):
                    s = b * T + jj * CH_
                    if jj % OPB == 0:
                        ob = obp.tile([128, 2, OBLK], f32, tag="ob")
                        project.ob = ob
                    pp = psp.tile([128, 2, CH_], f32, tag="pp")
                    for mo in range(2):
                        for k in range(2):
                            nc.tensor.matmul(
                                pp[:, mo, :], wp(k, mo), h[:, k, s:s + CH_],
                                start=(mo == 0 and k == 0),
                                stop=(mo == 1 and k == 1),
                            )
                    oo = (jj % OPB) * CH_
                    # bp folded into the activation's per-partition bias
                    for mo in range(2):
                        nc.scalar.activation(
                            project.ob[:, mo, oo:oo + CH_], pp[:, mo, :],
                            AF.Sigmoid,
                            bias=w2[:, PK2_BP + mo:PK2_BP + mo + 1])
                    if jj % OPB == OPB - 1:
                        s0 = b * T + (jj - (OPB - 1)) * CH_
                        nc.sync.dma_start(
                            out=outT_d[:, :, s0:s0 + OBLK], in_=project.ob[:])

            # ---- block stream: sweep 1 (staging) then sweeps 2..K ----
            pending = []   # [(b, rub_t, do_proj)] scans awaiting emission

            def flush(stage):
                """Emit one full scan of the previous block (kk = stage).
                Positioned after pair 0 / pair 2 of the current block so the
                DVE always has a fresh rh pair banked ahead of each scan."""
                if not pending:
                    return
                pb, prub, dp = pending[0]
                scan(pb, prub, stage, 0, T, 0.0)
                if stage == 1:
                    if dp:
                        project(pb, 0, CPB_)
                    pending.pop(0)

            def xprefetch(b, j):
                # consume the group prefetched one XBLK ago and prefetch the
                # next so chunk 0 never waits on DMA
                xc = xprefetch.nxt if b + j > 0 else xc0
                s0 = b * T + j * CH_ + XBLK
                if s0 < BLOC * T:
                    nxt = xcp.tile([128, 2, XBLK], bf16, tag="xc")
                    for k in range(2):
                        nc.sync.dma_start(
                            out=nxt[:, k, :], in_=xT_d[k, :, s0:s0 + XBLK])
                    xprefetch.nxt = nxt
                return xc

            for kiter in range(K_):
                s1 = (kiter == 0)
                last = (kiter == K_ - 1)
                for b in range(BLOC):
                    rub_t = rubp.tile([128, 4, T], bf16, tag="ru")
                    cb = csc.tile([128, 2, T], bf16, tag="c")
                    if s1:
                        for j in range(CPB_):
                            if j % PBLK == 0:
                                xc = xprefetch(b, j)
                            sweep1_chunk(b, j, xc, rub_t, cb)
                            if j == 2:
                                flush(0)
                            elif j == 6:
                                flush(1)
                    else:
                        for jp in range(CPB_ // 2):
                            gates_chunk(b, 2 * jp, rub_t)
                            gates_chunk(b, 2 * jp + 1, rub_t)
                            cand_pair(b, jp, rub_t, cb)
                            if jp == 0:
                                flush(0)
                            elif jp == 2:
                                flush(1)
                    zbatch(rub_t, cb)
                    pending.append((b, rub_t, last))

            # drain the final block: half scans with projection interleaved
            fb, frub, _ = pending.pop(0)
            hf = T // 2
            for kk in range(2):
                scan(fb, frub, kk, 0, hf, 0.0)
            project(fb, 0, CPB_ // 2)
            for kk in range(2):
                scan(fb, frub, kk, hf, T,
                     h[:, kk, fb * T + hf - 1:fb * T + hf])
            project(fb, CPB_ // 2, CPB_)

    nc.finalize()
    return nc


def _get_nc(K_, CH_):
    key = (K_, CH_)
    if key not in _cache:
        _cache[key] = _build(K_, CH_)
    return _cache[key]


def _pack_weights(Wg, bg, Wc, bc, Wp, bp):
    import ml_dtypes

    bf16 = ml_dtypes.bfloat16
    w1 = np.zeros((128, PKW1), dtype=bf16)
    w2 = np.zeros((128, PKW2), dtype=bf16)

    def put(w, off, a):  # a: [2, 128, X] -> cols [off : off + 2X]
        X = a.shape[2]
        for k in range(2):
            w[:, off + k * X:off + (k + 1) * X] = a[k].astype(bf16)

    put(w1, PK1_WGX, Wg[:256].reshape(2, 128, 512))
    put(w1, PK1_WCX, Wc[:256].reshape(2, 128, 256))
    w1[:, PK1_EYE:PK1_EYE + 128] = np.eye(128, dtype=np.float32).astype(bf16)
    w1[0:2, PK1_BG01:PK1_BG01 + 128] = bg[:256].reshape(2, 128).astype(bf16)
    w1[0:2, PK1_BG23:PK1_BG23 + 128] = bg[256:].reshape(2, 128).astype(bf16)
    w1[0:2, PK1_BC:PK1_BC + 128] = bc.reshape(2, 128).astype(bf16)
    for r in range(2):
        w1[r, PK1_MASK + r * CH:PK1_MASK + (r + 1) * CH] = bf16(1.0)
    put(w2, PK2_WGH, Wg[256:].reshape(2, 128, 512))
    put(w2, PK2_WCH, Wc[256:].reshape(2, 128, 256))
    put(w2, PK2_WP, Wp.reshape(2, 128, 256))
    w2[:, PK2_BP:PK2_BP + 2] = bp.reshape(2, 128).T.astype(bf16)
    return w1, w2


def run_gru(x, Wg, bg, Wc, bc, Wp, bp, K_=None, CH_=None, trace=False):
    from concourse.bass_utils import run_bass_kernel_spmd
    import ml_dtypes

    K_ = K_ or K
    CH_ = CH_ or CH
    x = np.asarray(x, dtype=np.float32)
    nc = _get_nc(K_, CH_)
    w1, w2 = _pack_weights(Wg, bg, Wc, bc, Wp, bp)
    in_maps = []
    for core in range(NCORES):
        x_core = x[core * BLOC:(core + 1) * BLOC]
        xT = np.ascontiguousarray(
            x_core.transpose(2, 0, 1).reshape(2, 128, N).astype(
                ml_dtypes.bfloat16))
        in_maps.append({"xT": xT, "wpk1": w1, "wpk2": w2})
    res = run_bass_kernel_spmd(nc, in_maps, list(range(NCORES)), trace=trace)
    outs = []
    for core in range(NCORES):
        oT = res.results[core]["outT"]  # [128, 2, N]
        o = (oT.reshape(128, 2, BLOC, T)
             .transpose(2, 3, 1, 0).reshape(BLOC, T, OUT))
        outs.append(o)
    full = np.concatenate(outs, axis=0).astype(np.float32)
    return full, res


def kernel(x, Wg, bg, Wc, bc, Wp, bp):
    out, _ = run_gru(
        np.asarray(x), np.asarray(Wg), np.asarray(bg), np.asarray(Wc),
        np.asarray(bc), np.asarray(Wp), np.asarray(bp),
    )
    return out
